# revision 23
# baseline (speedup 1.0000x reference)
"""Trainium2 Bass kernel for nn_DeltaEdgeModel (edge-attention GNN).

Strategy (8 NeuronCores, SPMD), v2:
  - Shard the E=4096 query-edge dim: 512 q-edges/core; replicate K/V.
  - fp8(e4m3) DoubleRow matmuls (0.5 cycles/row) for all projections:
    host fuses the node-context into the edge projections
      K = G@(Wn Wk) + ef@Wk + bk_eff,  V = G@(Wn Wv) + ef@Wv  (bias folded
    into bo via the softmax ones-trick), so the intermediate x tensor is
    never materialized.  fp8 weights are pre-scaled by 32 on the host
    (avoids e4m3 denormals); the 1/32 rides the psum-drain scale.
  - Scores stay bf16 (two heads packed in the PE via tile_position, exact
    baseline pattern); exp on ScalarE goes PSUM -> fp8 SBUF directly.
  - attn@v is an fp8 DoubleRow matmul over kt-PAIRS: lhsT v8[:,t,:,h,:]
    ([128,2,66], ones column at 64 gives the softmax denominator), rhs
    p8[:,2,512] = the exp output of two consecutive k-tiles.  Two k-tiles
    per instruction at 0.5 cycles/row = 4x the bf16 per-kt rate.
  - Multiplicative {0,1} adjacency mask (fp8, halves its DMA bytes) applied
    post-exp, alternating DVE / GpSimd so neither trails the ACT exp rate.
  - 1/denominator via DVE `divide` + a 1-contraction PE matmul broadcast:
    no Ln/Exp round-trip, zero activation-table switches mid-kernel.
  - Residual stream exact fp32 (local slices), as in v1.
  - One fp8 AllGather (Shared-HBM output) exchanges layer-1 outputs; the
    gathered o1 feeds layer-2 K/V directly as a second DoubleRow term.
  - K/V production is software-pipelined INTO the attention k-loop (blk b
    unlocks ktpair 2b) so ScalarE's exp stream starts ~1.5us after the
    layer inputs (DMA chunks / the gather) land.
  - Output stays [NCLS, QL] on device; host transposes.
"""

import sys
import os

for _p in ("/opt/trn_rl_repo", "/root/.axon_site/_ro/trn_rl_repo"):
    if os.path.isdir(_p) and _p not in sys.path:
        sys.path.insert(0, _p)

import numpy as np
import ml_dtypes

import concourse.bass as bass
import concourse.bacc as bacc
import concourse.mybir as mybir
import concourse.tile as tile
from concourse.bass_utils import run_bass_kernel_spmd

BF16 = ml_dtypes.bfloat16
FP8 = ml_dtypes.float8_e4m3
F32 = mybir.dt.float32
BF = mybir.dt.bfloat16
F8 = mybir.dt.float8e4
AF = mybir.ActivationFunctionType
DR = mybir.MatmulPerfMode.DoubleRowSwInterleave
ALU = mybir.AluOpType

N_CORES = 8
N_NODES, E = 1024, 4096
D = 256          # edge dim
H = 4            # heads
HD = 64          # head dim
NCLS = 16
QL = E // N_CORES          # local query edges per core = 512
KT = E // 128              # k tiles = 32
KTP = KT // 2              # kt pairs = 16
BLK = E // 512             # 512-col production blocks = 8
WS = 32.0                  # host pre-scale on fp8 weights (psum drains 1/32)
SQ = 1.0 / np.sqrt(HD)     # folded into Wq/bq on host
USE_DIVIDE = False          # DVE divide for 1/denom (else Ln+Exp on ACT)
USE_POOL = False            # GpSimd for mask-muls / psum drains
RB_SBUF = True            # stage the denom broadcast through SBUF


# --------------------------------------------------------------------------
# device program
# --------------------------------------------------------------------------

def build_nc():
    nc = bacc.Bacc("TRN2", target_bir_lowering=False, debug=False,
                   num_devices=N_CORES)

    def din(name, shape, dt=F32):
        return nc.dram_tensor(name, shape, dt, kind="ExternalInput")

    # host-pre-laid-out inputs (per core)
    g8 = din("g8", [128, 2, E], F8)            # G^T c-tiles (G = nf[src]||nf[dst])
    ef8 = din("ef8", [128, 2, E], F8)          # edge_features^T c-tiles
    ef_loc = din("ef_loc", [128, 2, QL])       # fp32 local slice (residual)
    g_loc = din("g_loc", [128, 2, QL], BF)
    mask8 = din("mask8", [128, KTP, 2, QL], F8)  # adjacency {0,1}
    wn_bf = [din(f"w_n{l}", [128, 2, D], BF) for l in (1, 2)]
    bn = [din(f"b_n{l}", [128, 2]) for l in (1, 2)]
    wq_bf = [din(f"w_q{l}", [128, 2, D], BF) for l in (1, 2)]
    bq = [din(f"b_q{l}", [128, 2]) for l in (1, 2)]
    wkn8 = [din(f"w_kn{l}", [128, 2, 128, 2], F8) for l in (1, 2)]
    wk8 = [din(f"w_k{l}", [128, 2, 128, 2], F8) for l in (1, 2)]
    bk = [din(f"b_k{l}", [128, 2]) for l in (1, 2)]
    wnv8 = [din(f"w_nv{l}", [128, 2, D], F8) for l in (1, 2)]
    wv8 = [din(f"w_v{l}", [128, 2, D], F8) for l in (1, 2)]
    wo = [din(f"w_o{l}", [64, H, D], BF) for l in (1, 2)]
    bo = [din(f"b_o{l}", [128, 2]) for l in (1, 2)]
    id_f = din("id_f", [128, 128])
    wc1 = din("w_c1", [128, 2, D])
    bc1 = din("b_c1", [128, 2])
    wc2 = din("w_c2", [128, 2, NCLS])
    bc2 = din("b_c2", [NCLS, 1])

    out = nc.dram_tensor("out", [NCLS, QL], F32, kind="ExternalOutput")

    with tile.TileContext(nc) as tc:
        with (
            tc.tile_pool(name="const", bufs=1) as cp,
            tc.tile_pool(name="work", bufs=1) as wp,
            tc.tile_pool(name="ppool", bufs=4) as ppool,
            tc.tile_pool(name="psproj", bufs=4, space="PSUM") as pp,
            tc.tile_pool(name="psscore", bufs=2, space="PSUM") as pss,
            tc.tile_pool(name="dram", bufs=1, space="DRAM") as dp,
        ):
            _late_dmas = []
            _gate_insts = []

            def load(dram, shape, dt=F32, eng=None, late=False):
                t = cp.tile(shape, dt, tag=f"c_{dram.name}")
                inst = (eng or nc.sync).dma_start(t[:], dram[:])
                if late:
                    _late_dmas.append(inst)
                return t

            # ---- startup-critical loads, striped across the 3 DMA queues
            wn_s = [load(wn_bf[0], [128, 2, D], BF, nc.sync)]
            bn_s = [load(bn[0], [128, 2], F32, nc.sync)]
            ef_loc_s = load(ef_loc, [128, 2, QL], F32, nc.sync)
            wq_s = [load(wq_bf[0], [128, 2, D], BF, nc.scalar)]
            bq_s = [load(bq[0], [128, 2], F32, nc.scalar)]
            g_loc_s = load(g_loc, [128, 2, QL], BF, nc.scalar)
            id_f_s = load(id_f, [128, 128], F32, nc.scalar)
            wkn_s = [load(wkn8[0], [128, 2, 128, 2], F8, nc.gpsimd)]
            wk_s = [load(wk8[0], [128, 2, 128, 2], F8, nc.gpsimd)]
            bk_s = [load(bk[0], [128, 2], F32, nc.gpsimd)]
            wnv_s = [load(wnv8[0], [128, 2, D], F8, nc.gpsimd)]
            wv_s = [load(wv8[0], [128, 2, D], F8, nc.gpsimd)]
            # big activations: 8 column-chunks striped over queues
            g8_s = cp.tile([128, 2, E], F8, tag="c_g8")
            ef8_s = cp.tile([128, 2, E], F8, tag="c_ef8")
            _dma_engs = [nc.sync, nc.scalar, nc.gpsimd]
            _ch_insts = []
            for c in range(BLK):
                sl = slice(c * 512, c * 512 + 512)
                _ch_insts.append(_dma_engs[(2 * c) % 3].dma_start(
                    g8_s[:, :, sl], g8[:, :, sl]))
                _ch_insts.append(_dma_engs[(2 * c + 1) % 3].dma_start(
                    ef8_s[:, :, sl], ef8[:, :, sl]))
            _gate_insts = [_ch_insts[5], _ch_insts[6]]
            # layer-2 + misc weights (gated behind the first input chunks)
            wn_s.append(load(wn_bf[1], [128, 2, D], BF, nc.sync, late=True))
            bn_s.append(load(bn[1], [128, 2], F32, nc.sync, late=True))
            wq_s.append(load(wq_bf[1], [128, 2, D], BF, nc.scalar, late=True))
            bq_s.append(load(bq[1], [128, 2], F32, nc.scalar, late=True))
            wkn_s.append(load(wkn8[1], [128, 2, 128, 2], F8, nc.gpsimd, late=True))
            wk_s.append(load(wk8[1], [128, 2, 128, 2], F8, nc.gpsimd, late=True))
            bk_s.append(load(bk[1], [128, 2], F32, nc.gpsimd, late=True))
            wnv_s.append(load(wnv8[1], [128, 2, D], F8, nc.sync, late=True))
            wv_s.append(load(wv8[1], [128, 2, D], F8, nc.sync, late=True))
            wo_s = [load(wo[0], [64, H, D], BF, nc.scalar, late=True),
                    load(wo[1], [64, H, D], BF, nc.scalar, late=True)]
            bo_s = [load(bo[0], [128, 2], F32, nc.gpsimd, late=True),
                    load(bo[1], [128, 2], F32, nc.gpsimd, late=True)]
            wc1_s = load(wc1, [128, 2, D], F32, nc.sync, late=True)
            bc1_s = load(bc1, [128, 2], F32, nc.sync, late=True)
            wc2_s = load(wc2, [128, 2, NCLS], F32, nc.sync, late=True)
            bc2_s = load(bc2, [NCLS, 1], F32, nc.sync, late=True)
            # adjacency mask: 8 ktpair-chunks, spread over queues, gated
            mask_s = cp.tile([128, KTP, 2, QL], F8, tag="c_mask")
            for c in range(8):
                sl = slice(c * 2, c * 2 + 2)
                _late_dmas.append(_dma_engs[c % 3].dma_start(
                    mask_s[:, sl], mask8[:, sl]))

            for _ld in _late_dmas:
                for _g in _gate_insts:
                    tile.add_dep_helper(_ld.ins, _g.ins, sync=True,
                                        reason="late input load")

            mm = nc.tensor.matmul

            # small constants: ones row at partition 64 (for 1/denom) and
            # ones lhsT column (for the PE partition-broadcast)
            ones_r = wp.tile([128, QL], BF, tag="ones_r")
            ones_c = wp.tile([128, HD], BF, tag="ones_c")
            nc.vector.memset(ones_r[64:65, :], 1.0)
            nc.vector.memset(ones_c[64:65, :], 1.0)

            def psum_drain(eng, dst, ps, bias=None, scale=None):
                """psum -> sbuf on a chosen engine. eng in {'act','dve','pool'}."""
                if eng == "act":
                    if bias is None and scale is None:
                        nc.scalar.copy(dst, ps)
                    elif scale is None:
                        nc.scalar.activation(dst, ps, AF.Identity, bias=bias)
                    else:
                        nc.scalar.activation(dst, ps, AF.Identity, bias=bias,
                                             scale=scale)
                elif eng == "dve":
                    if scale is not None and bias is not None:
                        nc.vector.tensor_scalar(dst, ps, scale, bias,
                                                op0=ALU.mult, op1=ALU.add)
                    elif scale is not None:
                        nc.vector.tensor_scalar_mul(dst, ps, scale)
                    elif bias is not None:
                        nc.vector.tensor_scalar_add(dst, ps, bias)
                    else:
                        nc.vector.tensor_copy(dst, ps)
                else:
                    e = nc.gpsimd if USE_POOL else nc.vector
                    if scale is not None and bias is not None:
                        e.tensor_scalar(dst, ps, scale, bias,
                                        op0=ALU.mult, op1=ALU.add)
                    elif scale is not None:
                        e.tensor_scalar_mul(dst, ps, scale)
                    elif bias is not None:
                        e.tensor_scalar_add(dst, ps, bias)
                    else:
                        e.tensor_copy(dst, ps)

            def xloc_q(l, res_loc):
                """fp32 local residual x and bf16 Q^T from local inputs.
                res_loc: fp32 [128,2,QL] AP fn dt -> AP (ef_loc / o1loc)."""
                xloc = wp.tile([128, 2, QL], F32, tag="xloc", name=f"xloc{l}")
                xloc_bf = wp.tile([128, 2, QL], BF, tag="xloc_bf",
                                  name=f"xloc_bf{l}")
                for dt in range(2):
                    dsl = slice(dt * 128, dt * 128 + 128)
                    ps = pp.tile([128, 512], F32, tag="proj")
                    mm(ps[:], wn_s[l][:, 0, dsl], g_loc_s[:, 0, :],
                       start=True, stop=False)
                    mm(ps[:], wn_s[l][:, 1, dsl], g_loc_s[:, 1, :],
                       start=False, stop=False)
                    mm(ps[:], id_f_s[:], res_loc(dt), start=False, stop=True)
                    psum_drain("act", xloc[:, dt, :], ps[:],
                               bias=bn_s[l][:, dt:dt + 1])
                    nc.vector.tensor_copy(xloc_bf[:, dt, :], xloc[:, dt, :])
                q_bf = wp.tile([128, 2, QL], BF, tag="q_bf", name=f"q_bf{l}")
                for dt in range(2):
                    dsl = slice(dt * 128, dt * 128 + 128)
                    ps = pp.tile([128, 512], F32, tag="proj")
                    mm(ps[:], wq_s[l][:, 0, dsl], xloc_bf[:, 0, :],
                       start=True, stop=False)
                    mm(ps[:], wq_s[l][:, 1, dsl], xloc_bf[:, 1, :],
                       start=False, stop=True)
                    psum_drain("act", q_bf[:, dt, :], ps[:],
                               bias=bq_s[l][:, dt:dt + 1])
                return xloc, q_bf

            def layer(l, src2, src2_insts, xloc, q_bf):
                """One edge-attention layer.
                src2: fp8 [128,2,E] second production operand (ef8 / o1g8).
                src2_insts: per-blk list of instructions that produced src2
                  cols (for nothing — deps are via the tile framework).
                Returns fp32 local out [128,2,QL]."""
                k_bf = wp.tile([128, 2, E], BF, tag="k_bf")
                # v8: SwInterleave weights layout for the attn@v DoubleRow:
                # free = [pos jj, kt-in-pair i]; out-col j = 127-jj. jj 64..
                # hold host-reversed V columns (out cols 63..0), jj=63 the
                # ones (denominator -> out col 64), jj<63 zero pads.
                v8 = wp.tile([128, KTP, H, 128, 2], F8, tag="v8")
                nc.vector.memset(v8[:, :, :, 0:HD, :], 0.0)
                nc.vector.memset(v8[:, :, :, HD - 1:HD, :], 1.0)
                aon = wp.tile([64, H, QL], BF, tag="aon")

                def prod(b):
                    """K columns + V rows for blk b (512 edges)."""
                    bsl = slice(b * 512, b * 512 + 512)
                    for dt in range(2):
                        ps = pp.tile([128, 512], F32, tag="proj")
                        mm(ps[:], wkn_s[l][:, dt, :, :], g8_s[:, :, bsl],
                           start=True, stop=False, perf_mode=DR)
                        mm(ps[:], wk_s[l][:, dt, :, :], src2[:, :, bsl],
                           start=False, stop=True, perf_mode=DR)
                        psum_drain("pool" if dt == 0 else "dve",
                                   k_bf[:, dt, bsl], ps[:],
                                   bias=bk_s[l][:, dt:dt + 1], scale=1.0 / WS)
                    # V: two e-tiles (=one ktpair) per psum bank, plain fp8
                    for tp in (2 * b, 2 * b + 1):
                        ps = pp.tile([128, 512], F32, tag="proj")
                        for half in range(2):
                            esl = slice(tp * 256 + half * 128,
                                        tp * 256 + half * 128 + 128)
                            osl = slice(half * 256, half * 256 + 256)
                            mm(ps[:, osl], g8_s[:, 0, esl], wnv_s[l][:, 0, :],
                               start=(half == 0), stop=False)
                            mm(ps[:, osl], g8_s[:, 1, esl], wnv_s[l][:, 1, :],
                               start=False, stop=False)
                            mm(ps[:, osl], src2[:, 0, esl], wv_s[l][:, 0, :],
                               start=False, stop=False)
                            mm(ps[:, osl], src2[:, 1, esl], wv_s[l][:, 1, :],
                               start=False, stop=(half == 1))
                        psum_drain("pool",
                                   v8[:, tp, :, HD:128, :],
                                   ps[:].rearrange("p (i h d) -> p h d i",
                                                   i=2, h=H),
                                   scale=1.0 / WS)

                def pass_heads(pair, produce):
                    pav = [pp.tile([128, 512], F32, tag="proj",
                                   name=f"pav{l}_{pair}_{hh}") for hh in range(2)]
                    sc_d = {}
                    p8_d = {}
                    for t in range(KTP + 1):
                        if t < KTP:
                            if produce:
                                if t == 0:
                                    prod(0)
                                if t % 2 == 0 and t // 2 + 1 < BLK:
                                    prod(t // 2 + 1)
                            # scores for both heads of the pair, kt = 2t, 2t+1
                            for hh in range(2):
                                psl = slice(hh * 64, hh * 64 + 64)
                                sc = pss.tile([128, 2, 512], F32, tag="sc",
                                              name=f"sc{hh}")
                                for i in range(2):
                                    ksl = slice((2 * t + i) * 128,
                                                (2 * t + i) * 128 + 128)
                                    mm(sc[:, i, :], k_bf[psl, pair, ksl],
                                       q_bf[psl, pair, :], start=True, stop=True,
                                       tile_position=(hh * 64, 0))
                                p8 = ppool.tile([128, 2, QL], F8, tag="p8",
                                                name=f"p8_{hh}")
                                nc.scalar.activation(p8[:], sc[:], AF.Exp)
                                mul = nc.vector.tensor_mul if (
                                    hh == 0 or not USE_POOL) else \
                                    nc.gpsimd.tensor_mul
                                mul(p8[:], p8[:], mask_s[:, t, :, :])
                                sc_d[hh], p8_d[hh] = sc, p8
                        if t > 0:
                            # attn@v for ktpair t-1 (software-pipelined by one)
                            for hh in range(2):
                                h = 2 * pair + hh
                                mm(pav[hh][:, :],
                                   v8[:, t - 1, h, :, :], p8_prev[hh][:],
                                   start=(t - 1 == 0), stop=(t - 1 == KTP - 1),
                                   perf_mode=DR)
                        p8_prev = dict(p8_d)
                    # 1/denominator (DVE divide), PE broadcast, normalize
                    for hh in range(2):
                        h = 2 * pair + hh
                        rcp = wp.tile([128, QL], BF, tag="rcp", bufs=2,
                                      name=f"rcp{hh}")
                        if USE_DIVIDE:
                            nc.vector.tensor_tensor(
                                rcp[64:65, :], ones_r[64:65, :],
                                pav[hh][64:65, :], op=ALU.divide)
                        else:
                            nc.scalar.activation(rcp[64:65, :],
                                                 pav[hh][64:65, :], AF.Ln)
                            nc.scalar.activation(rcp[64:65, :],
                                                 rcp[64:65, :], AF.Exp,
                                                 scale=-1.0)
                        rb = pp.tile([128, 512], F32, tag="proj",
                                     name=f"rb{hh}")
                        mm(rb[0:HD, :], ones_c[64:65, :], rcp[64:65, :],
                           start=True, stop=True, tile_position=(64, 0))
                        if RB_SBUF:
                            rbs = wp.tile([128, QL], F32, tag="rbs", bufs=2,
                                          name=f"rbs{hh}")
                            nc.scalar.copy(rbs[0:HD, :], rb[0:HD, :])
                            nc.vector.tensor_mul(aon[0:HD, h, :],
                                                 pav[hh][0:HD, :], rbs[0:HD, :])
                        else:
                            nc.vector.tensor_mul(aon[0:HD, h, :],
                                                 pav[hh][0:HD, :], rb[0:HD, :])

                pass_heads(0, True)
                pass_heads(1, False)

                # ---- y = aon @ Wo + bo_eff + xloc (residual) ----
                oloc = wp.tile([128, 2, QL], F32, tag=f"oloc{l}")
                for et in range(2):
                    esl = slice(et * 128, et * 128 + 128)
                    ps = pp.tile([128, 512], F32, tag="proj")
                    for h in range(H):
                        mm(ps[:], wo_s[l][0:HD, h, esl], aon[0:HD, h, :],
                           start=(h == 0), stop=False)
                    mm(ps[:], id_f_s[:], xloc[:, et, :], start=False, stop=True)
                    psum_drain("act", oloc[:, et, :], ps[:],
                               bias=bo_s[l][:, et:et + 1])
                return oloc

            # ============ layer 1 ============
            xloc1, q1 = xloc_q(0, lambda dt: ef_loc_s[:, dt, :])
            o1loc = layer(0, ef8_s, None, xloc1, q1)

            # ============ exchange: single fp8 AllGather of o1 ============
            o1f8 = wp.tile([128, 2, QL], F8, tag="o1f8")
            for dt in range(2):
                nc.vector.tensor_copy(o1f8[:, dt, :], o1loc[:, dt, :])
            ci = dp.tile([128, 2 * QL], F8, name="cc_in")
            co = dp.tile([N_CORES, 128, 2 * QL], F8, name="cc_out",
                         addr_space="Shared")
            nc.sync.dma_start(ci[:], o1f8[:].rearrange("p i q -> p (i q)"))
            nc.gpsimd.collective_compute(
                "AllGather",
                mybir.AluOpType.bypass,
                replica_groups=[list(range(N_CORES))],
                ins=[ci[:].opt()],
                outs=[co[:].opt()],
            )
            # o1 gathered, transposed-tile layout [128, 2, E] (fp8)
            o1g8 = wp.tile([128, 2, E], F8, tag="o1g8")
            for c in range(N_CORES):
                gsl = slice(c * QL, (c + 1) * QL)
                _dma_engs[c % 3].dma_start(
                    o1g8[:, :, gsl],
                    co[c].rearrange("p (i q) -> p i q", i=2))

            # hoisted into the collective's dead time: local residual + Q2
            xloc2, q2 = xloc_q(1, lambda dt: o1loc[:, dt, :])
            o2loc = layer(1, o1g8, None, xloc2, q2)

            # ============ classifier ============
            h_s = wp.tile([128, 2, QL], F32, tag="h")
            for dt in range(2):
                dsl = slice(dt * 128, dt * 128 + 128)
                ps = pp.tile([128, 512], F32, tag="proj")
                mm(ps[:], wc1_s[:, 0, dsl], o2loc[:, 0, :], start=True, stop=False)
                mm(ps[:], wc1_s[:, 1, dsl], o2loc[:, 1, :], start=False, stop=True)
                nc.scalar.activation(h_s[:, dt, :], ps[:], AF.Gelu,
                                     bias=bc1_s[:, dt:dt + 1])
            ps_l = pp.tile([128, 512], F32, tag="proj")
            mm(ps_l[0:NCLS, :], wc2_s[:, 0, :], h_s[:, 0, :], start=True, stop=False)
            mm(ps_l[0:NCLS, :], wc2_s[:, 1, :], h_s[:, 1, :], start=False, stop=True)
            lg = wp.tile([NCLS, QL], F32, tag="lg")
            nc.vector.tensor_scalar_add(lg[:], ps_l[0:NCLS, :], bc2_s[:, 0:1])
            nc.sync.dma_start(out[:], lg[:])

    nc.compile()
    return nc


# --------------------------------------------------------------------------
# host-side data prep
# --------------------------------------------------------------------------

def _tiles_T(a):
    """[E, D2] array -> transposed tile layout [128, D2//128, E]."""
    d2 = a.shape[1]
    return np.ascontiguousarray(
        a.T.reshape(d2 // 128, 128, a.shape[0]).transpose(1, 0, 2))


def _wtile(w):
    """[G, D] weight -> [128, G//128, D] (lhsT tiles, partition=contraction)."""
    g, d = w.shape
    return np.ascontiguousarray(w.reshape(g // 128, 128, d).transpose(1, 0, 2))


def _btile(b):
    return np.ascontiguousarray(b.reshape(-1, 128).T)  # [128, 2]


def _wtile_swi(w):
    """[256, 256] weight -> SwInterleave lhsT layout [128, 2, 128, 2]:
    [p, d-slice s, pos jj, c-tile i] = w[p + 128 i, 128 s + 127 - jj]."""
    arr = w.reshape(2, 128, 2, 128)        # [i, p, s, dd]
    arr = arr[:, :, :, ::-1]               # dd -> jj (column reversal)
    return np.ascontiguousarray(arr.transpose(1, 2, 3, 0))


def _wtile_vrev(w):
    """[256, 256] V weight with per-head reversed output columns, tiled."""
    w2 = np.ascontiguousarray(
        w.reshape(w.shape[0], H, HD)[:, :, ::-1].reshape(w.shape[0], -1))
    return _wtile(w2)


def prep_in_maps(inputs):
    f32 = np.float32
    nf = np.asarray(inputs["node_features"], f32)
    ef = np.asarray(inputs["edge_features"], f32)
    ei = np.asarray(inputs["edge_index"], np.int32)
    src, dst = ei[0], ei[1]

    G = np.concatenate([nf[src], nf[dst]], axis=1)            # [E, 256]
    g_t = _tiles_T(G)
    ef_t = _tiles_T(ef)                                       # [128, 2, E] f32

    adj = ((src[:, None] == src[None, :]) | (src[:, None] == dst[None, :]) |
           (dst[:, None] == src[None, :]) | (dst[:, None] == dst[None, :]))
    # [128, KTP, 2, E]: partition = edge%128, ktpair, kt-in-pair
    adj_t = adj.reshape(KTP, 2, 128, E).transpose(2, 0, 1, 3)

    com = {
        "g8": g_t.astype(FP8),
        "ef8": ef_t.astype(FP8),
        "id_f": np.eye(128, dtype=f32),
    }
    for l, pre in ((1, "a1"), (2, "a2")):
        Wn = np.asarray(inputs[f"{pre}_Wn"], f32)
        bn = np.asarray(inputs[f"{pre}_bn"], f32)
        Wq = np.asarray(inputs[f"{pre}_Wq"], f32)
        bq = np.asarray(inputs[f"{pre}_bq"], f32)
        Wk = np.asarray(inputs[f"{pre}_Wk"], f32)
        bk = np.asarray(inputs[f"{pre}_bk"], f32)
        Wv = np.asarray(inputs[f"{pre}_Wv"], f32)
        bv = np.asarray(inputs[f"{pre}_bv"], f32)
        Wo = np.asarray(inputs[f"{pre}_Wo"], f32)
        bo = np.asarray(inputs[f"{pre}_bo"], f32)
        com[f"w_n{l}"] = _wtile(Wn).astype(BF16)
        com[f"b_n{l}"] = _btile(bn)
        com[f"w_q{l}"] = _wtile(Wq * SQ).astype(BF16)
        com[f"b_q{l}"] = _btile(bq * SQ)
        com[f"w_kn{l}"] = _wtile_swi(Wn @ Wk * WS).astype(FP8)
        com[f"w_k{l}"] = _wtile_swi(Wk * WS).astype(FP8)
        com[f"b_k{l}"] = _btile(bn @ Wk + bk)
        com[f"w_nv{l}"] = _wtile_vrev(Wn @ Wv * WS).astype(FP8)
        com[f"w_v{l}"] = _wtile_vrev(Wv * WS).astype(FP8)
        # [64, H, D]: head h rows at partitions 0:64
        com[f"w_o{l}"] = np.ascontiguousarray(
            Wo.reshape(H, HD, D).transpose(1, 0, 2)).astype(BF16)
        # attn rows sum to 1 => the (bv + bn@Wv) value shift passes through
        bo_eff = bo + (bv + bn @ Wv) @ Wo
        com[f"b_o{l}"] = _btile(bo_eff)
    com["w_c1"] = _wtile(np.asarray(inputs["cls_W1"], f32))
    com["b_c1"] = _btile(np.asarray(inputs["cls_b1"], f32))
    com["w_c2"] = _wtile(np.asarray(inputs["cls_W2"], f32))
    com["b_c2"] = np.asarray(inputs["cls_b2"], f32).reshape(NCLS, 1)

    in_maps = []
    for c in range(N_CORES):
        q = slice(c * QL, (c + 1) * QL)
        m = dict(com)
        m["ef_loc"] = np.ascontiguousarray(ef_t[:, :, q])
        m["g_loc"] = np.ascontiguousarray(g_t[:, :, q]).astype(BF16)
        m["mask8"] = np.ascontiguousarray(adj_t[:, :, :, q]).astype(FP8)
        in_maps.append(m)
    return in_maps


def assemble_out(res):
    return np.concatenate(
        [np.asarray(res.results[c]["out"]).T for c in range(N_CORES)], axis=0)


_NC_CACHE = None


def kernel(**inputs) -> np.ndarray:
    global _NC_CACHE
    in_maps = prep_in_maps(inputs)
    if _NC_CACHE is None:
        _NC_CACHE = build_nc()
    res = run_bass_kernel_spmd(_NC_CACHE, in_maps, core_ids=list(range(N_CORES)))
    return assemble_out(res)


# revision 44
# speedup vs baseline: 1.0027x; 1.0027x over previous
"""Trainium2 Bass kernel for nn_DeltaEdgeModel (edge-attention GNN).

Strategy (8 NeuronCores, SPMD), v2:
  - Shard the E=4096 query-edge dim: 512 q-edges/core; replicate K/V.
  - fp8(e4m3) DoubleRow matmuls (0.5 cycles/row) for all projections:
    host fuses the node-context into the edge projections
      K = G@(Wn Wk) + ef@Wk + bk_eff,  V = G@(Wn Wv) + ef@Wv  (bias folded
    into bo via the softmax ones-trick), so the intermediate x tensor is
    never materialized.  fp8 weights are pre-scaled by 32 on the host
    (avoids e4m3 denormals); the 1/32 rides the psum-drain scale.
  - Scores stay bf16 (two heads packed in the PE via tile_position, exact
    baseline pattern); exp on ScalarE goes PSUM -> fp8 SBUF directly.
  - attn@v is an fp8 DoubleRow matmul over kt-PAIRS: lhsT v8[:,t,:,h,:]
    ([128,2,66], ones column at 64 gives the softmax denominator), rhs
    p8[:,2,512] = the exp output of two consecutive k-tiles.  Two k-tiles
    per instruction at 0.5 cycles/row = 4x the bf16 per-kt rate.
  - Multiplicative {0,1} adjacency mask (fp8, halves its DMA bytes) applied
    post-exp, alternating DVE / GpSimd so neither trails the ACT exp rate.
  - 1/denominator via DVE `divide` + a 1-contraction PE matmul broadcast:
    no Ln/Exp round-trip, zero activation-table switches mid-kernel.
  - Residual stream exact fp32 (local slices), as in v1.
  - One fp8 AllGather (Shared-HBM output) exchanges layer-1 outputs; the
    gathered o1 feeds layer-2 K/V directly as a second DoubleRow term.
  - K/V production is software-pipelined INTO the attention k-loop (blk b
    unlocks ktpair 2b) so ScalarE's exp stream starts ~1.5us after the
    layer inputs (DMA chunks / the gather) land.
  - Output stays [NCLS, QL] on device; host transposes.
"""

import sys
import os

for _p in ("/opt/trn_rl_repo", "/root/.axon_site/_ro/trn_rl_repo"):
    if os.path.isdir(_p) and _p not in sys.path:
        sys.path.insert(0, _p)

import numpy as np
import ml_dtypes

import concourse.bass as bass
import concourse.bacc as bacc
import concourse.mybir as mybir
import concourse.tile as tile
from concourse.bass_utils import run_bass_kernel_spmd

BF16 = ml_dtypes.bfloat16
FP8 = ml_dtypes.float8_e4m3
F32 = mybir.dt.float32
BF = mybir.dt.bfloat16
F8 = mybir.dt.float8e4
AF = mybir.ActivationFunctionType
DR = mybir.MatmulPerfMode.DoubleRowSwInterleave
ALU = mybir.AluOpType

N_CORES = 8
N_NODES, E = 1024, 4096
D = 256          # edge dim
H = 4            # heads
HD = 64          # head dim
NCLS = 16
QL = E // N_CORES          # local query edges per core = 512
KT = E // 128              # k tiles = 32
KTP = KT // 2              # kt pairs = 16
BLK = E // 512             # 512-col production blocks = 8
WS = 32.0                  # host pre-scale on fp8 weights (psum drains 1/32)
SQ = 1.0 / np.sqrt(HD)     # folded into Wq/bq on host
USE_DIVIDE = True          # DVE divide for 1/denom (else Ln+Exp on ACT)
USE_POOL = True            # GpSimd for mask-muls / psum drains
RB_SBUF = True            # stage the denom broadcast through SBUF


# --------------------------------------------------------------------------
# device program
# --------------------------------------------------------------------------

def build_nc():
    nc = bacc.Bacc("TRN2", target_bir_lowering=False, debug=False,
                   num_devices=N_CORES)

    def din(name, shape, dt=F32):
        return nc.dram_tensor(name, shape, dt, kind="ExternalInput")

    # host-pre-laid-out inputs (per core)
    g8 = din("g8", [128, 2, E], F8)            # G^T c-tiles (G = nf[src]||nf[dst])
    ef8 = din("ef8", [128, 2, E], F8)          # edge_features^T c-tiles
    ef_loc = din("ef_loc", [128, 2, QL])       # fp32 local slice (residual)
    g_loc = din("g_loc", [128, 2, QL], BF)
    mask8 = din("mask8", [128, KTP, 2, QL], F8)  # adjacency {0,1}
    wn_bf = [din(f"w_n{l}", [128, 2, D], BF) for l in (1, 2)]
    bn = [din(f"b_n{l}", [128, 2]) for l in (1, 2)]
    wq_bf = [din(f"w_q{l}", [128, 2, D], BF) for l in (1, 2)]
    bq = [din(f"b_q{l}", [128, 2]) for l in (1, 2)]
    wn8 = [din(f"w_n8{l}", [128, 2, 128, 2], F8) for l in (1, 2)]
    wk8 = [din(f"w_k{l}", [128, 2, 128, 2], F8) for l in (1, 2)]
    wv8 = [din(f"w_v{l}", [128, 2, D], F8) for l in (1, 2)]
    id8 = din("id8", [128, 2, 128, 2], F8)     # 32*I256, SwInterleave tiles
    wo = [din(f"w_o{l}", [64, H, D], BF) for l in (1, 2)]
    bo = [din(f"b_o{l}", [128, 2]) for l in (1, 2)]
    id_f = din("id_f", [128, 128])
    wc1 = din("w_c1", [128, 2, D])
    bc1 = din("b_c1", [128, 2])
    wc2 = din("w_c2", [128, 2, NCLS])
    bc2 = din("b_c2", [NCLS, 1])

    out = nc.dram_tensor("out", [NCLS, QL], F32, kind="ExternalOutput")

    with tile.TileContext(nc) as tc:
        with (
            tc.tile_pool(name="const", bufs=1) as cp,
            tc.tile_pool(name="work", bufs=1) as wp,
            tc.tile_pool(name="ppool", bufs=4) as ppool,
            tc.tile_pool(name="psproj", bufs=4, space="PSUM") as pp,
            tc.tile_pool(name="psscore", bufs=2, space="PSUM") as pss,
            tc.tile_pool(name="dram", bufs=1, space="DRAM") as dp,
        ):
            _late_dmas = []
            _gate_insts = []

            def load(dram, shape, dt=F32, eng=None, late=False):
                t = cp.tile(shape, dt, tag=f"c_{dram.name}")
                inst = (eng or nc.sync).dma_start(t[:], dram[:])
                if late:
                    _late_dmas.append(inst)
                return t

            # ---- startup-critical loads, striped across the 3 DMA queues
            wn_s = [load(wn_bf[0], [128, 2, D], BF, nc.sync)]
            bn_s = [load(bn[0], [128, 2], F32, nc.sync)]
            ef_loc_s = load(ef_loc, [128, 2, QL], F32, nc.sync)
            wq_s = [load(wq_bf[0], [128, 2, D], BF, nc.scalar)]
            bq_s = [load(bq[0], [128, 2], F32, nc.scalar)]
            g_loc_s = load(g_loc, [128, 2, QL], BF, nc.scalar)
            id_f_s = load(id_f, [128, 128], F32, nc.scalar)
            wn8_s = [load(wn8[0], [128, 2, 128, 2], F8, nc.gpsimd)]
            wk_s = [load(wk8[0], [128, 2, 128, 2], F8, nc.gpsimd)]
            id8_s = load(id8, [128, 2, 128, 2], F8, nc.gpsimd)
            wv_s = [load(wv8[0], [128, 2, D], F8, nc.gpsimd)]
            # big activations: 8 column-chunks striped over queues
            g8_s = cp.tile([128, 2, E], F8, tag="c_g8")
            ef8_s = cp.tile([128, 2, E], F8, tag="c_ef8")
            _dma_engs = [nc.sync, nc.scalar, nc.gpsimd]
            _ch_insts = []
            for c in range(BLK):
                sl = slice(c * 512, c * 512 + 512)
                _ch_insts.append(_dma_engs[(2 * c) % 3].dma_start(
                    g8_s[:, :, sl], g8[:, :, sl]))
                _ch_insts.append(_dma_engs[(2 * c + 1) % 3].dma_start(
                    ef8_s[:, :, sl], ef8[:, :, sl]))
            _gate_insts = [_ch_insts[5], _ch_insts[6]]
            # layer-2 + misc weights (gated behind the first input chunks)
            wn_s.append(load(wn_bf[1], [128, 2, D], BF, nc.sync, late=True))
            bn_s.append(load(bn[1], [128, 2], F32, nc.sync, late=True))
            wq_s.append(load(wq_bf[1], [128, 2, D], BF, nc.scalar, late=True))
            bq_s.append(load(bq[1], [128, 2], F32, nc.scalar, late=True))
            wn8_s.append(load(wn8[1], [128, 2, 128, 2], F8, nc.gpsimd, late=True))
            wk_s.append(load(wk8[1], [128, 2, 128, 2], F8, nc.gpsimd, late=True))
            wv_s.append(load(wv8[1], [128, 2, D], F8, nc.sync, late=True))
            wo_s = [load(wo[0], [64, H, D], BF, nc.scalar, late=True),
                    load(wo[1], [64, H, D], BF, nc.scalar, late=True)]
            bo_s = [load(bo[0], [128, 2], F32, nc.gpsimd, late=True),
                    load(bo[1], [128, 2], F32, nc.gpsimd, late=True)]
            wc1_s = load(wc1, [128, 2, D], F32, nc.sync, late=True)
            bc1_s = load(bc1, [128, 2], F32, nc.sync, late=True)
            wc2_s = load(wc2, [128, 2, NCLS], F32, nc.sync, late=True)
            bc2_s = load(bc2, [NCLS, 1], F32, nc.sync, late=True)
            # adjacency mask: 8 ktpair-chunks, spread over queues, gated
            mask_s = cp.tile([128, KTP, 2, QL], F8, tag="c_mask")
            for c in range(8):
                sl = slice(c * 2, c * 2 + 2)
                _late_dmas.append(_dma_engs[c % 3].dma_start(
                    mask_s[:, sl], mask8[:, sl]))

            for _ld in _late_dmas:
                for _g in _gate_insts:
                    tile.add_dep_helper(_ld.ins, _g.ins, sync=True,
                                        reason="late input load")

            mm = nc.tensor.matmul

            # small constants: ones row at partition 64 (for 1/denom) and
            # ones lhsT column (for the PE partition-broadcast)
            ones_r = wp.tile([128, QL], BF, tag="ones_r")
            ones_c = wp.tile([128, HD], BF, tag="ones_c")
            nc.vector.memset(ones_r[64:65, :], 1.0)
            nc.vector.memset(ones_c[64:65, :], 1.0)

            def psum_drain(eng, dst, ps, bias=None, scale=None):
                """psum -> sbuf on a chosen engine. eng in {'act','dve','pool'}."""
                if eng == "act":
                    if bias is None and scale is None:
                        nc.scalar.copy(dst, ps)
                    elif scale is None:
                        nc.scalar.activation(dst, ps, AF.Identity, bias=bias)
                    else:
                        nc.scalar.activation(dst, ps, AF.Identity, bias=bias,
                                             scale=scale)
                elif eng == "dve":
                    if scale is not None and bias is not None:
                        nc.vector.tensor_scalar(dst, ps, scale, bias,
                                                op0=ALU.mult, op1=ALU.add)
                    elif scale is not None:
                        nc.vector.tensor_scalar_mul(dst, ps, scale)
                    elif bias is not None:
                        nc.vector.tensor_scalar_add(dst, ps, bias)
                    else:
                        nc.vector.tensor_copy(dst, ps)
                else:
                    e = nc.gpsimd if USE_POOL else nc.vector
                    if scale is not None and bias is not None:
                        e.tensor_scalar(dst, ps, scale, bias,
                                        op0=ALU.mult, op1=ALU.add)
                    elif scale is not None:
                        e.tensor_scalar_mul(dst, ps, scale)
                    elif bias is not None:
                        e.tensor_scalar_add(dst, ps, bias)
                    else:
                        e.tensor_copy(dst, ps)

            def xloc_q(l, res_loc):
                """fp32 local residual x and bf16 Q^T from local inputs.
                res_loc: fp32 [128,2,QL] AP fn dt -> AP (ef_loc / o1loc)."""
                xloc = wp.tile([128, 2, QL], F32, tag="xloc", name=f"xloc{l}")
                xloc_bf = wp.tile([128, 2, QL], BF, tag="xloc_bf",
                                  name=f"xloc_bf{l}")
                for dt in range(2):
                    dsl = slice(dt * 128, dt * 128 + 128)
                    ps = pp.tile([128, 512], F32, tag="proj", bufs=4)
                    mm(ps[:], wn_s[l][:, 0, dsl], g_loc_s[:, 0, :],
                       start=True, stop=False)
                    mm(ps[:], wn_s[l][:, 1, dsl], g_loc_s[:, 1, :],
                       start=False, stop=False)
                    mm(ps[:], id_f_s[:], res_loc(dt), start=False, stop=True)
                    psum_drain("act", xloc[:, dt, :], ps[:],
                               bias=bn_s[l][:, dt:dt + 1])
                    nc.vector.tensor_copy(xloc_bf[:, dt, :], xloc[:, dt, :])
                q_bf = wp.tile([128, 2, QL], BF, tag="q_bf", name=f"q_bf{l}")
                for dt in range(2):
                    dsl = slice(dt * 128, dt * 128 + 128)
                    ps = pp.tile([128, 512], F32, tag="proj", bufs=4)
                    mm(ps[:], wq_s[l][:, 0, dsl], xloc_bf[:, 0, :],
                       start=True, stop=False)
                    mm(ps[:], wq_s[l][:, 1, dsl], xloc_bf[:, 1, :],
                       start=False, stop=True)
                    psum_drain("act", q_bf[:, dt, :], ps[:],
                               bias=bq_s[l][:, dt:dt + 1])
                return xloc, q_bf

            # v8: SwInterleave weights layout for the attn@v DoubleRow:
            # free = [pos jj, kt-in-pair i]; out-col j = 127-jj. jj 64..
            # hold host-reversed V columns (out cols 63..0), jj=63 the
            # ones (denominator -> out col 64), jj<63 zero pads.  Pads are
            # set once here; only jj>=64 is rewritten per layer.
            v8 = wp.tile([128, KTP, H, 128, 2], F8, tag="v8")
            nc.vector.memset(v8[:, :, :, 0:HD, :], 0.0)
            nc.vector.memset(v8[:, :, :, HD - 1:HD, :], 1.0)

            def layer(l, src2, xloc, q_bf):
                """One edge-attention layer.
                src2: fp8 [128,2,E] second x-production operand (ef8 / o1g8).
                Returns fp32 local out [128,2,QL]."""
                x8 = wp.tile([128, 2, E], F8, tag="x8")
                k_bf = wp.tile([128, 2, E], BF, tag="k_bf")
                aon = wp.tile([64, H, QL], BF, tag="aon")

                def xprod(b):
                    """x = G@Wn + bn + src2, fp8, for blk b (512 edges)."""
                    bsl = slice(b * 512, b * 512 + 512)
                    for dt in range(2):
                        ps = pp.tile([128, 512], F32, tag="proj", bufs=4)
                        mm(ps[:], wn8_s[l][:, dt, :, :], g8_s[:, :, bsl],
                           start=True, stop=False, perf_mode=DR)
                        mm(ps[:], id8_s[:, dt, :, :], src2[:, :, bsl],
                           start=False, stop=True, perf_mode=DR)
                        psum_drain("dve", x8[:, dt, bsl], ps[:],
                                   bias=bn_s[l][:, dt:dt + 1], scale=1.0 / WS)

                def kvprod(b):
                    """K columns + V rows for blk b from x8.  The 32x weight
                    prescale stays in k_bf / v8 (folded into Wq / Wo on the
                    host) so the drains are pure copies on the DVE."""
                    bsl = slice(b * 512, b * 512 + 512)
                    for dt in range(2):
                        ps = pp.tile([128, 512], F32, tag="proj", bufs=4)
                        mm(ps[:], wk_s[l][:, dt, :, :], x8[:, :, bsl],
                           start=True, stop=True, perf_mode=DR)
                        psum_drain("dve", k_bf[:, dt, bsl], ps[:])
                    # V: two e-tiles (=one ktpair) per psum bank, plain fp8
                    for tp in (2 * b, 2 * b + 1):
                        ps = pp.tile([128, 512], F32, tag="proj", bufs=4)
                        for half in range(2):
                            esl = slice(tp * 256 + half * 128,
                                        tp * 256 + half * 128 + 128)
                            osl = slice(half * 256, half * 256 + 256)
                            mm(ps[:, osl], x8[:, 0, esl], wv_s[l][:, 0, :],
                               start=(half == 0), stop=False)
                            mm(ps[:, osl], x8[:, 1, esl], wv_s[l][:, 1, :],
                               start=False, stop=(half == 1))
                        psum_drain("dve",
                                   v8[:, tp, :, HD:128, :],
                                   ps[:].rearrange("p (i h d) -> p h d i",
                                                   i=2, h=H))

                def pass_heads(pair, produce):
                    pav = [pp.tile([128, 512], F32, tag="proj", bufs=4,
                                   name=f"pav{l}_{pair}_{hh}") for hh in range(2)]
                    if produce:
                        xprod(0)
                        xprod(1)
                        kvprod(0)
                    sc_d = {}
                    p8_d = {}
                    for t in range(KTP + 1):
                        if t < KTP:
                            if produce and t % 2 == 1:
                                if t // 2 + 2 < BLK:
                                    xprod(t // 2 + 2)
                                if t // 2 + 1 < BLK:
                                    kvprod(t // 2 + 1)
                            # scores for both heads of the pair, kt = 2t, 2t+1
                            for hh in range(2):
                                psl = slice(hh * 64, hh * 64 + 64)
                                sc = pss.tile([128, 2, 512], F32, tag="sc",
                                              name=f"sc{hh}")
                                for i in range(2):
                                    ksl = slice((2 * t + i) * 128,
                                                (2 * t + i) * 128 + 128)
                                    mm(sc[:, i, :], k_bf[psl, pair, ksl],
                                       q_bf[psl, pair, :], start=True, stop=True,
                                       tile_position=(hh * 64, 0))
                                p8 = ppool.tile([128, 2, QL], F8, tag="p8",
                                                name=f"p8_{hh}")
                                nc.scalar.activation(p8[:], sc[:], AF.Exp)
                                mul = nc.gpsimd.tensor_mul if (
                                    USE_POOL and hh == 1) else \
                                    nc.vector.tensor_mul
                                mul(p8[:], p8[:], mask_s[:, t, :, :])
                                sc_d[hh], p8_d[hh] = sc, p8
                        if t > 0:
                            # attn@v for ktpair t-1 (software-pipelined by one)
                            for hh in range(2):
                                h = 2 * pair + hh
                                mm(pav[hh][:, :],
                                   v8[:, t - 1, h, :, :], p8_prev[hh][:],
                                   start=(t - 1 == 0), stop=(t - 1 == KTP - 1),
                                   perf_mode=DR)
                        p8_prev = dict(p8_d)
                    return pav

                pav4 = pass_heads(0, True) + pass_heads(1, False)
                # 1/denominator for all 4 heads at once: one Ln / Exp table
                # round-trip per layer, then a 1-contraction PE matmul
                # broadcast and the normalizing multiply.
                rcp = wp.tile([128, H, QL], BF, tag="rcp")
                for h in range(H):
                    nc.scalar.activation(rcp[64:65, h, :],
                                         pav4[h][64:65, :], AF.Ln)
                for h in range(H):
                    nc.scalar.activation(rcp[64:65, h, :],
                                         rcp[64:65, h, :], AF.Exp, scale=-1.0)
                for h in range(H):
                    rb = pss.tile([128, 2, 512], F32, tag="sc",
                                  name=f"rb{h}")
                    mm(rb[0:HD, 0, :], ones_c[64:65, :], rcp[64:65, h, :],
                       start=True, stop=True, tile_position=(64, 0))
                    rbs = wp.tile([128, QL], F32, tag="rbs", bufs=2,
                                  name=f"rbs{h}")
                    nc.scalar.copy(rbs[0:HD, :], rb[0:HD, 0, :])
                    nc.vector.tensor_mul(aon[0:HD, h, :],
                                         pav4[h][0:HD, :], rbs[0:HD, :])

                # ---- y = aon @ Wo + bo_eff + xloc (residual) ----
                oloc = wp.tile([128, 2, QL], F32, tag=f"oloc{l}")
                for et in range(2):
                    esl = slice(et * 128, et * 128 + 128)
                    ps = pp.tile([128, 512], F32, tag="proj", bufs=4)
                    for h in range(H):
                        mm(ps[:], wo_s[l][0:HD, h, esl], aon[0:HD, h, :],
                           start=(h == 0), stop=False)
                    mm(ps[:], id_f_s[:], xloc[:, et, :], start=False, stop=True)
                    psum_drain("act", oloc[:, et, :], ps[:],
                               bias=bo_s[l][:, et:et + 1])
                return oloc

            # ============ layer 1 ============
            xloc1, q1 = xloc_q(0, lambda dt: ef_loc_s[:, dt, :])
            o1loc = layer(0, ef8_s, xloc1, q1)

            # ============ exchange: single fp8 AllGather of o1 ============
            o1f8 = wp.tile([128, 2, QL], F8, tag="o1f8")
            for dt in range(2):
                nc.vector.tensor_copy(o1f8[:, dt, :], o1loc[:, dt, :])
            ci = dp.tile([128, 2 * QL], F8, name="cc_in")
            co = dp.tile([N_CORES, 128, 2 * QL], F8, name="cc_out",
                         addr_space="Shared")
            nc.sync.dma_start(ci[:], o1f8[:].rearrange("p i q -> p (i q)"))
            nc.gpsimd.collective_compute(
                "AllGather",
                mybir.AluOpType.bypass,
                replica_groups=[list(range(N_CORES))],
                ins=[ci[:].opt()],
                outs=[co[:].opt()],
            )
            # o1 gathered, transposed-tile layout [128, 2, E] (fp8)
            o1g8 = wp.tile([128, 2, E], F8, tag="o1g8")
            for c in range(N_CORES):
                gsl = slice(c * QL, (c + 1) * QL)
                _dma_engs[c % 3].dma_start(
                    o1g8[:, :, gsl],
                    co[c].rearrange("p (i q) -> p i q", i=2))

            # hoisted into the collective's dead time: local residual + Q2
            xloc2, q2 = xloc_q(1, lambda dt: o1loc[:, dt, :])
            o2loc = layer(1, o1g8, xloc2, q2)

            # ============ classifier ============
            h_s = wp.tile([128, 2, QL], F32, tag="h")
            for dt in range(2):
                dsl = slice(dt * 128, dt * 128 + 128)
                ps = pp.tile([128, 512], F32, tag="proj", bufs=4)
                mm(ps[:], wc1_s[:, 0, dsl], o2loc[:, 0, :], start=True, stop=False)
                mm(ps[:], wc1_s[:, 1, dsl], o2loc[:, 1, :], start=False, stop=True)
                nc.scalar.activation(h_s[:, dt, :], ps[:], AF.Gelu,
                                     bias=bc1_s[:, dt:dt + 1])
            ps_l = pp.tile([128, 512], F32, tag="proj", bufs=4)
            mm(ps_l[0:NCLS, :], wc2_s[:, 0, :], h_s[:, 0, :], start=True, stop=False)
            mm(ps_l[0:NCLS, :], wc2_s[:, 1, :], h_s[:, 1, :], start=False, stop=True)
            lg = wp.tile([NCLS, QL], F32, tag="lg")
            nc.vector.tensor_scalar_add(lg[:], ps_l[0:NCLS, :], bc2_s[:, 0:1])
            nc.sync.dma_start(out[:], lg[:])

    nc.compile()
    return nc


# --------------------------------------------------------------------------
# host-side data prep
# --------------------------------------------------------------------------

def _tiles_T(a):
    """[E, D2] array -> transposed tile layout [128, D2//128, E]."""
    d2 = a.shape[1]
    return np.ascontiguousarray(
        a.T.reshape(d2 // 128, 128, a.shape[0]).transpose(1, 0, 2))


def _wtile(w):
    """[G, D] weight -> [128, G//128, D] (lhsT tiles, partition=contraction)."""
    g, d = w.shape
    return np.ascontiguousarray(w.reshape(g // 128, 128, d).transpose(1, 0, 2))


def _btile(b):
    return np.ascontiguousarray(b.reshape(-1, 128).T)  # [128, 2]


def _wtile_swi(w):
    """[256, 256] weight -> SwInterleave lhsT layout [128, 2, 128, 2]:
    [p, d-slice s, pos jj, c-tile i] = w[p + 128 i, 128 s + 127 - jj]."""
    arr = w.reshape(2, 128, 2, 128)        # [i, p, s, dd]
    arr = arr[:, :, :, ::-1]               # dd -> jj (column reversal)
    return np.ascontiguousarray(arr.transpose(1, 2, 3, 0))


def _wtile_vrev(w):
    """[256, 256] V weight with per-head reversed output columns, tiled."""
    w2 = np.ascontiguousarray(
        w.reshape(w.shape[0], H, HD)[:, :, ::-1].reshape(w.shape[0], -1))
    return _wtile(w2)


def prep_in_maps(inputs):
    f32 = np.float32
    nf = np.asarray(inputs["node_features"], f32)
    ef = np.asarray(inputs["edge_features"], f32)
    ei = np.asarray(inputs["edge_index"], np.int32)
    src, dst = ei[0], ei[1]

    G = np.concatenate([nf[src], nf[dst]], axis=1)            # [E, 256]
    g_t = _tiles_T(G)
    ef_t = _tiles_T(ef)                                       # [128, 2, E] f32

    adj = ((src[:, None] == src[None, :]) | (src[:, None] == dst[None, :]) |
           (dst[:, None] == src[None, :]) | (dst[:, None] == dst[None, :]))
    # [128, KTP, 2, E]: partition = edge%128, ktpair, kt-in-pair
    adj_t = adj.reshape(KTP, 2, 128, E).transpose(2, 0, 1, 3)

    com = {
        "g8": g_t.astype(FP8),
        "ef8": ef_t.astype(FP8),
        "id_f": np.eye(128, dtype=f32),
        "id8": _wtile_swi(np.eye(D, dtype=f32) * WS).astype(FP8),
    }
    for l, pre in ((1, "a1"), (2, "a2")):
        Wn = np.asarray(inputs[f"{pre}_Wn"], f32)
        bn = np.asarray(inputs[f"{pre}_bn"], f32)
        Wq = np.asarray(inputs[f"{pre}_Wq"], f32)
        bq = np.asarray(inputs[f"{pre}_bq"], f32)
        Wk = np.asarray(inputs[f"{pre}_Wk"], f32)
        bk = np.asarray(inputs[f"{pre}_bk"], f32)
        Wv = np.asarray(inputs[f"{pre}_Wv"], f32)
        bv = np.asarray(inputs[f"{pre}_bv"], f32)
        Wo = np.asarray(inputs[f"{pre}_Wo"], f32)
        bo = np.asarray(inputs[f"{pre}_bo"], f32)
        com[f"w_n{l}"] = _wtile(Wn).astype(BF16)
        com[f"b_n{l}"] = _btile(bn)
        # k_bf carries the 32x weight prescale; 1/32 rides in Wq.  bk is
        # dropped entirely: it adds a per-query constant to every score,
        # which softmax cancels.
        com[f"w_q{l}"] = _wtile(Wq * (SQ / WS)).astype(BF16)
        com[f"b_q{l}"] = _btile(bq * (SQ / WS))
        com[f"w_n8{l}"] = _wtile_swi(Wn * WS).astype(FP8)
        com[f"w_k{l}"] = _wtile_swi(Wk * WS).astype(FP8)
        com[f"w_v{l}"] = _wtile_vrev(Wv * WS).astype(FP8)
        # [64, H, D]: head h rows at partitions 0:64; v8 carries the 32x
        # prescale (numerator only, the ones column is unscaled), so Wo
        # absorbs the 1/32.
        com[f"w_o{l}"] = np.ascontiguousarray(
            Wo.reshape(H, HD, D).transpose(1, 0, 2) / WS).astype(BF16)
        # attn rows sum to 1 => the bv value shift passes through to bo
        com[f"b_o{l}"] = _btile(bo + bv @ Wo)
    com["w_c1"] = _wtile(np.asarray(inputs["cls_W1"], f32))
    com["b_c1"] = _btile(np.asarray(inputs["cls_b1"], f32))
    com["w_c2"] = _wtile(np.asarray(inputs["cls_W2"], f32))
    com["b_c2"] = np.asarray(inputs["cls_b2"], f32).reshape(NCLS, 1)

    in_maps = []
    for c in range(N_CORES):
        q = slice(c * QL, (c + 1) * QL)
        m = dict(com)
        m["ef_loc"] = np.ascontiguousarray(ef_t[:, :, q])
        m["g_loc"] = np.ascontiguousarray(g_t[:, :, q]).astype(BF16)
        m["mask8"] = np.ascontiguousarray(adj_t[:, :, :, q]).astype(FP8)
        in_maps.append(m)
    return in_maps


def assemble_out(res):
    return np.concatenate(
        [np.asarray(res.results[c]["out"]).T for c in range(N_CORES)], axis=0)


_NC_CACHE = None


def kernel(**inputs) -> np.ndarray:
    global _NC_CACHE
    in_maps = prep_in_maps(inputs)
    if _NC_CACHE is None:
        _NC_CACHE = build_nc()
    res = run_bass_kernel_spmd(_NC_CACHE, in_maps, core_ids=list(range(N_CORES)))
    return assemble_out(res)
